# revision 34
# baseline (speedup 1.0000x reference)
"""Trainium2 Bass kernel for a dense transformer block (B=2,T=2048,E=1024,H=16,DH=64,FF=4096).

Sharding: tensor-parallel across 8 NeuronCores - core c computes attention heads
{2c, 2c+1} and FFN columns [512c, 512c+512), returns the transposed partial output
yT = attn_partial^T + ffn_partial^T  [E, B*T] in bf16; the host sums the 8 partials
(the all-reduce), adds the residual x and the output biases bo/b2.

LayerNorm runs on the host: the device receives xhat = (x-mean)/sqrt(var+eps)
pre-transposed [E, TN] in bf16; the LN gains are folded into the projection
weights (diag(g) @ W) and the LN shifts become per-output-channel bias columns
applied on the Activation engine during PSUM eviction (Identity+bias for Q/K/V,
Relu+bias for the FFN hidden).

All matmuls run in bf16 (fp32 PSUM accumulate). The FFN hidden u = relu(.) stays
resident in SBUF (no DRAM round trip). Attention: scores S^T[s,t] per head ->
exp on ACT (bf16 out) -> PV in S^T layout with a ones-column in V carrying the
partition function Z as PSUM row 64. Normalization transposes O to natural
layout on the PE (identity matmul), takes the reciprocal of the Z column on the
vector engine in [128,4] layout (fast, multi-partition), scales, and transposes
back - an engine-local chain with no DMA broadcasts, keeping the tensor engine
fed at iteration boundaries. The FFN down-projection W2^T u runs as a separate
weight-stationary phase before attention (one weight load per (group, k-tile)
streamed across all 8 token chunks into all 8 PSUM banks), leaving only the Wo
matmuls - interleaved into the attention stream as fillers - plus an add of the
precomputed FFN partial at output assembly. Both heads' scores share one PSUM
pair tile so a single exp activation covers them.
"""

import sys
import numpy as np

sys.path.insert(0, "/opt/trn_rl_repo")

from contextlib import ExitStack

import ml_dtypes

import concourse.bacc as bacc
import concourse.bass as bass
import concourse.tile as tile
from concourse import mybir
from concourse.bass_utils import run_bass_kernel_spmd

B, T, E, H, DH, FF = 2, 2048, 1024, 16, 64, 4096
NCORES = 8
TN = B * T            # 4096 tokens total
NH = H // NCORES      # 2 heads per core
FFC = FF // NCORES    # 512 ffn cols per core
TCH = 512             # token chunk
NCH = TN // TCH       # 8 chunks
SC = 128              # s-chunk for attention
NSC = TN // SC        # 32 s-chunks
EK = E // 128         # 8 contraction chunks over E
EPS = 1e-5
LA = 3                # PV lookahead (s-chunks) behind exp

F32 = mybir.dt.float32
BF16 = mybir.dt.bfloat16
FP8 = mybir.dt.float8e4
AF = mybir.ActivationFunctionType
OP = mybir.AluOpType
PM = mybir.MatmulPerfMode
NPBF16 = ml_dtypes.bfloat16


def _bcast(ap, nparts):
    """Partition-broadcast view of a [1, N] ap -> [nparts, N]."""
    return bass.AP(tensor=ap.tensor, offset=ap.offset, ap=[[0, nparts]] + list(ap.ap[-1:]))


def _build_device_program():
    nc = bacc.Bacc()

    xT = nc.dram_tensor("xT", [E, TN], BF16, kind="ExternalInput")
    wqkv = nc.dram_tensor("wqkv", [E, 3 * NH * DH], BF16, kind="ExternalInput")
    w1 = nc.dram_tensor("w1", [E, FFC], BF16, kind="ExternalInput")
    w2 = nc.dram_tensor("w2", [FFC, E], BF16, kind="ExternalInput")
    wo = nc.dram_tensor("wo", [NH * DH, E], BF16, kind="ExternalInput")
    qkvbias = nc.dram_tensor("qkvbias", [128, 3], F32, kind="ExternalInput")
    ubias = nc.dram_tensor("ubias", [128, FFC // 128], F32, kind="ExternalInput")
    ident_in = nc.dram_tensor("ident", [128, 128], BF16, kind="ExternalInput")
    yT = nc.dram_tensor("yT", [E, TN], BF16, kind="ExternalOutput")

    xTr = xT.rearrange("(c p) t -> c p t", p=128)       # [8, 128, TN]
    yTr = yT.rearrange("(g p) t -> g p t", p=128)       # [8, 128, TN]

    with tile.TileContext(nc) as tc, ExitStack() as top:
        const = top.enter_context(tc.tile_pool(name="const", bufs=1))
        wpool = top.enter_context(tc.tile_pool(name="wpool", bufs=1))
        big = top.enter_context(tc.tile_pool(name="big", bufs=1))

        ident = const.tile([128, 128], BF16)
        nc.gpsimd.dma_start(out=ident, in_=ident_in[:, :])
        qkvb = const.tile([128, 3], F32)
        nc.gpsimd.dma_start(out=qkvb, in_=qkvbias[:, :])
        ub = const.tile([128, FFC // 128], F32)
        nc.gpsimd.dma_start(out=ub, in_=ubias[:, :])

        # resident weights; wqkv rides the sync queue interleaved with the
        # first x chunk (both gate the first matmuls), w1 on gpsimd
        wqkv_sb = wpool.tile([128, EK, 3 * NH * DH], BF16)
        w1_sb = wpool.tile([128, EK, FFC], BF16)
        for c in range(EK):
            wq1 = nc.gpsimd if c % 2 == 0 else nc.scalar
            wq1.dma_start(out=w1_sb[:, c, :], in_=w1[c * 128:(c + 1) * 128, :])
        w2_sb = wpool.tile([128, FFC // 128, E], BF16)
        wo_sb = wpool.tile([128, E], BF16)

        # resident activations
        QT = big.tile([NH * DH, TN], BF16)       # [128, 4096]
        KT = big.tile([NH * DH, TN], BF16)
        # V natural + ones col; padded to DH+2 so per-(sc,h) slices start at
        # 4-byte-aligned SBUF offsets (65*2B would misalign the weight loads)
        V = big.tile([128, NSC, NH, DH + 2], BF16)
        OT = big.tile([NH * DH, TN], BF16)
        U = big.tile([128, FFC // 128, TN], BF16)   # relu(f@W1+b1)^T resident
        YP = big.tile([128, EK, TN], BF16)          # W2^T u partial, per e-group

        # ---------------- Phase AB: projections, per token chunk -------------
        with ExitStack() as ab, \
             tc.tile_pool(name="xs", bufs=2) as xs_pool, \
             tc.tile_pool(name="abwork", bufs=2) as work, \
             tc.tile_pool(name="mm_ps", bufs=6, space="PSUM") as mm_ps, \
             tc.tile_pool(name="vt_ps", bufs=2, space="PSUM") as vt_ps:
            for t in range(NCH):
                ts0, ts1 = t * TCH, (t + 1) * TCH
                xs = xs_pool.tile([128, EK, TCH], BF16, tag="xs")
                for c in range(EK):
                    if t == 0:
                        nc.sync.dma_start(out=wqkv_sb[:, c, :],
                                          in_=wqkv[c * 128:(c + 1) * 128, :])
                    xq = nc.gpsimd if (t > 0 and c % 2 == 1) else nc.sync
                    xq.dma_start(out=xs[:, c, :], in_=xTr[c, :, ts0:ts1])

                # QKV^T projections
                for g in range(3):
                    ps = mm_ps.tile([128, TCH], F32, tag="mm")
                    gs = slice(g * 128, (g + 1) * 128)
                    for c in range(EK):
                        nc.tensor.matmul(ps, wqkv_sb[:, c, gs], xs[:, c, :],
                                         start=(c == 0), stop=(c == EK - 1))
                    if g == 0:
                        nc.scalar.activation(out=QT[:, ts0:ts1], in_=ps,
                                             func=AF.Identity, bias=qkvb[:, 0:1])
                    elif g == 1:
                        nc.scalar.activation(out=KT[:, ts0:ts1], in_=ps,
                                             func=AF.Identity, bias=qkvb[:, 1:2])
                    else:
                        vt_tmp = work.tile([128, TCH], BF16, tag="vt_tmp")
                        nc.scalar.activation(out=vt_tmp, in_=ps,
                                             func=AF.Identity, bias=qkvb[:, 2:3])
                        # transpose V^T -> V natural, per 128-token block
                        for j in range(TCH // 128):
                            pvt = vt_ps.tile([128, 128], BF16, tag="pvt")
                            nc.tensor.transpose(pvt, vt_tmp[:, j * 128:(j + 1) * 128], ident)
                            sc = t * (TCH // 128) + j
                            nc.vector.tensor_copy(
                                out=V[:, sc, :, 0:DH],
                                in_=pvt.rearrange("p (h d) -> p h d", h=NH))

                # FFN up: u^T = relu(psum + bias), resident in SBUF
                for g in range(FFC // 128):
                    ps = mm_ps.tile([128, TCH], F32, tag="mm")
                    gs = slice(g * 128, (g + 1) * 128)
                    for c in range(EK):
                        nc.tensor.matmul(ps, w1_sb[:, c, gs], xs[:, c, :],
                                         start=(c == 0), stop=(c == EK - 1))
                    nc.scalar.activation(out=U[:, g, ts0:ts1], in_=ps,
                                         func=AF.Relu, bias=ub[:, g:g + 1])

        # deferred CD-only loads (transfer during the projection phase)
        nc.gpsimd.memset(V[:, :, :, DH:DH + 1], 1.0)
        for k in range(FFC // 128):
            nc.gpsimd.dma_start(out=w2_sb[:, k, :], in_=w2[k * 128:(k + 1) * 128, :])
        nc.gpsimd.dma_start(out=wo_sb, in_=wo[:, :])

        # ---------------- Phase B2: yp = W2^T u, weight-stationary ----------
        # one weight load per (g,k), streamed across all 8 token chunks into
        # all 8 PSUM banks (32 LDWEIGHTS total instead of 256 in the old
        # attention-interleaved form)
        with ExitStack() as b2, \
             tc.tile_pool(name="yp_ps", bufs=8, space="PSUM") as yp_ps:
            for g in range(EK - 1):
                gs = slice(g * 128, (g + 1) * 128)
                tiles = [yp_ps.tile([128, TCH], F32, tag="yp", name="ps_yp")
                         for _ in range(NCH)]
                for k in range(FFC // 128):
                    for t in range(NCH):
                        nc.tensor.matmul(tiles[t], w2_sb[:, k, gs],
                                         U[:, k, t * TCH:(t + 1) * TCH],
                                         start=(k == 0),
                                         stop=(k == FFC // 128 - 1))
                for t in range(NCH):
                    eng = nc.scalar if t % 2 == 0 else nc.vector
                    if t % 2 == 0:
                        nc.scalar.activation(out=YP[:, g, t * TCH:(t + 1) * TCH],
                                             in_=tiles[t], func=AF.Copy)
                    else:
                        nc.vector.tensor_copy(out=YP[:, g, t * TCH:(t + 1) * TCH],
                                              in_=tiles[t])

        # ---------------- Phase CD: attention + output, per (batch, t-chunk) --
        with ExitStack() as cd, \
             tc.tile_pool(name="expp", bufs=10) as expp, \
             tc.tile_pool(name="cdwork", bufs=2) as cdw, \
             tc.tile_pool(name="zp", bufs=2) as zp, \
             tc.tile_pool(name="s_ps", bufs=2, space="PSUM") as s_ps, \
             tc.tile_pool(name="o_ps", bufs=1, space="PSUM") as o_ps, \
             tc.tile_pool(name="zn_ps", bufs=1, space="PSUM") as zn_ps, \
             tc.tile_pool(name="y_ps", bufs=1, space="PSUM") as y_ps:
            nsc = T // SC

            def out_group(ts0, ts1, g, borrow=False):
                # y^T[gs, ts] = Wo^T O^T + precomputed W2 partial. In the
                # final drain the score banks are idle - borrow two as extra
                # accumulator slots so the adds pipeline instead of
                # serializing on the single y bank.
                if borrow and g % 3 != 0:
                    st = s_ps.tile([128, NH, TCH], F32, tag="s", name="ps_yb")
                    ps_y = st[:, 0, :]
                else:
                    ps_y = y_ps.tile([128, TCH], F32, tag="y", name="ps_y")
                gs = slice(g * 128, (g + 1) * 128)
                nc.tensor.matmul(ps_y, wo_sb[:, gs], OT[:, ts0:ts1],
                                 start=True, stop=True)
                y_sb = cdw.tile([128, TCH], BF16, tag="y_sb", name="y_sb")
                nc.vector.tensor_tensor(out=y_sb, in0=ps_y,
                                        in1=YP[:, g, ts0:ts1], op=OP.add)
                nc.sync.dma_start(out=yTr[g, :, ts0:ts1], in_=y_sb)

            def b2_tail_task(t):
                # group 7's W2 partial for chunk t, displaced from B2 into
                # iteration 0's otherwise-empty filler slots
                a, z = t * TCH, (t + 1) * TCH
                gs = slice((EK - 1) * 128, EK * 128)
                ps_y = y_ps.tile([128, TCH], F32, tag="y", name="ps_y")
                for k in range(FFC // 128):
                    nc.tensor.matmul(ps_y, w2_sb[:, k, gs], U[:, k, a:z],
                                     start=(k == 0), stop=(k == FFC // 128 - 1))
                if t % 2 == 0:
                    nc.scalar.activation(out=YP[:, EK - 1, a:z], in_=ps_y,
                                         func=AF.Copy)
                else:
                    nc.vector.tensor_copy(out=YP[:, EK - 1, a:z], in_=ps_y)

            # deferred tasks popped into the attention stream; starts with
            # group 7's displaced B2 work during iteration 0
            pending = [(lambda t=t: b2_tail_task(t)) for t in range(NCH)]
            for b in range(B):
                for tq in range(T // TCH):
                    ts0 = b * T + tq * TCH
                    ts1 = ts0 + TCH

                    ps_o = o_ps.tile([DH + 1, NH, TCH], F32, tag="o", name="ps_o")
                    exs = {}
                    # scores+exp LA waves ahead of PV; previous iter's OUT
                    # groups interleave so the PE stays dense during exps
                    for sc in range(nsc + LA):
                        # PV and filler tasks run FIRST: when the next score
                        # pair's bank wait (exp of sc-2) would block, the PE
                        # executes them during the wait instead of stalling
                        # the whole in-order queue behind the scores
                        if sc >= LA:
                            psc = sc - LA
                            gpsc = b * nsc + psc
                            ex = exs.pop(psc)
                            for h in range(NH):
                                nc.tensor.matmul(ps_o[:, h, :],
                                                 V[:, gpsc, h, 0:DH + 1],
                                                 ex[:, h, :],
                                                 start=(psc == 0),
                                                 stop=(psc == nsc - 1))
                        if pending and sc >= 2:
                            pending.pop(0)()
                        if sc < nsc:
                            gsc = b * nsc + sc
                            ps_s = s_ps.tile([128, NH, TCH], F32, tag="s",
                                             name="ps_s")
                            for h in range(NH):
                                hs = slice(h * DH, (h + 1) * DH)
                                nc.tensor.matmul(
                                    ps_s[:, h, :], KT[hs, gsc * SC:(gsc + 1) * SC],
                                    QT[hs, ts0:ts1], start=True, stop=True)
                            ex = expp.tile([128, NH, TCH], BF16, tag="ex", name="ex")
                            nc.scalar.activation(out=ex, in_=ps_s, func=AF.Exp,
                                                 scale=float(DH) ** -0.5)
                            exs[sc] = ex

                    while pending:
                        pending.pop(0)()

                    # Z-normalization, split into deferred tasks that
                    # interleave into the NEXT iteration's attention stream
                    # (the PE would otherwise idle behind the chain's DVE
                    # steps at every iteration boundary). Only the ou
                    # evictions run now - they release the PV accumulator.
                    ous = []
                    for h in range(NH):
                        ou = cdw.tile([DH + 1, TCH], BF16, tag=f"ou{h}", name="ou")
                        nc.vector.tensor_copy(out=ou, in_=ps_o[:, h, :])
                        ous.append(ou)

                    def zn_front(h, ou):
                        znat = zn_ps.tile([128, TCH], BF16, tag="zn", name="znat")
                        znat_v = znat[:, 0:(TCH // 128) * (DH + 2)].rearrange(
                            "p (j d) -> p j d", d=DH + 2)
                        for j in range(TCH // 128):
                            nc.tensor.transpose(
                                znat_v[:, j, 0:DH + 1], ou[:, j * 128:(j + 1) * 128],
                                ident[0:DH + 1, 0:DH + 1])
                        zinv = zp.tile([128, TCH // 128, 1], F32, tag="zinv",
                                       name="zinv")
                        nc.vector.reciprocal(out=zinv, in_=znat_v[:, :, DH:DH + 1])
                        onat = cdw.tile([128, TCH // 128, DH], BF16, tag="onat",
                                        name="onat")
                        for j in range(TCH // 128):
                            nc.vector.tensor_scalar(
                                out=onat[:, j, :], in0=znat_v[:, j, 0:DH],
                                scalar1=zinv[:, j, :], scalar2=None, op0=OP.mult)
                        return onat

                    def zn_back(h, onat, a, z):
                        ot_ps = zn_ps.tile([128, TCH], BF16, tag="zn", name="ot_ps")
                        for j in range(TCH // 128):
                            nc.tensor.transpose(
                                ot_ps[0:DH, j * 128:(j + 1) * 128], onat[:, j, :],
                                ident)
                        nc.vector.tensor_copy(out=OT[h * DH:(h + 1) * DH, a:z],
                                              in_=ot_ps[0:DH, :])

                    state = {}

                    def mk_front(h, ou):
                        def run():
                            state[h] = zn_front(h, ou)
                        return run

                    def mk_back(h, a, z):
                        def run():
                            zn_back(h, state.pop(h), a, z)
                        return run

                    pending = [mk_front(0, ous[0]), mk_front(1, ous[1]),
                               mk_back(0, ts0, ts1), mk_back(1, ts0, ts1)]
                    last = (b == B - 1 and tq == T // TCH - 1)
                    pending += [
                        (lambda g=g, a=ts0, z=ts1, w=last: out_group(a, z, g, w))
                        for g in range(EK)]

            while pending:
                pending.pop(0)()

    nc.finalize()
    return nc


_CACHE = {}


def _get_program():
    if "nc" not in _CACHE:
        _CACHE["nc"] = _build_device_program()
    return _CACHE["nc"]


def _host_prepare(x, Wq, Wk, Wv, Wo, bo, W1, b1, W2, b2, g1, be1, g2, be2):
    xf = np.ascontiguousarray(np.asarray(x, np.float32).reshape(TN, E))
    mu = xf.mean(axis=1, keepdims=True)
    xc = xf - mu
    var = np.mean(xc * xc, axis=1, keepdims=True)
    xhat = xc / np.sqrt(var + EPS)
    xhT = np.ascontiguousarray(xhat.T.astype(NPBF16))

    Wq, Wk, Wv = (np.asarray(w, np.float32) for w in (Wq, Wk, Wv))
    Wo, W1, W2 = (np.asarray(w, np.float32) for w in (Wo, W1, W2))
    g1, be1, g2, be2 = (np.asarray(v, np.float32) for v in (g1, be1, g2, be2))
    b1 = np.asarray(b1, np.float32)

    in_maps = []
    for c in range(NCORES):
        hs = [NH * c + i for i in range(NH)]

        def qkv_block(W):
            Wc = W[hs]                                   # [NH, E, DH]
            Wp = (g1[None, :, None] * Wc)                # diag(g1) @ W
            main = np.transpose(Wp, (1, 0, 2)).reshape(E, NH * DH)
            bias = np.einsum("e,hed->hd", be1, Wc).reshape(NH * DH)
            return main, bias

        qm, qb = qkv_block(Wq)
        km, kb = qkv_block(Wk)
        vm, vb = qkv_block(Wv)
        wqkv = np.concatenate([qm, km, vm], axis=1)
        qkvb = np.stack([qb, kb, vb], axis=1)            # [128, 3]

        J = slice(FFC * c, FFC * (c + 1))
        W1c = W1[:, J]
        w1m = g2[:, None] * W1c
        bu = (be2 @ W1c + b1[J]).reshape(FFC // 128, 128).T  # [128, 4]

        in_maps.append({
            "xT": xhT,
            "wqkv": np.ascontiguousarray(wqkv.astype(NPBF16)),
            "w1": np.ascontiguousarray(w1m.astype(NPBF16)),
            "w2": np.ascontiguousarray(W2[J, :].astype(NPBF16)),
            "wo": np.ascontiguousarray(
                Wo[NH * DH * c: NH * DH * (c + 1), :].astype(NPBF16)),
            "qkvbias": np.ascontiguousarray(qkvb, np.float32),
            "ubias": np.ascontiguousarray(bu, np.float32),
            "ident": np.eye(128, dtype=NPBF16),
        })
    return xf, in_maps


def _host_finish(x, bo, b2, xf, results):
    acc = xf.copy()
    for res in results:
        acc += np.asarray(res["yT"], np.float32).T
    acc += np.asarray(bo, np.float32)[None, :] + np.asarray(b2, np.float32)[None, :]
    return acc.reshape(np.asarray(x).shape).astype(np.float32)


def kernel(x, Wq, Wk, Wv, Wo, bo, W1, b1, W2, b2, g1, be1, g2, be2, _trace=False):
    nc = _get_program()
    xf, in_maps = _host_prepare(x, Wq, Wk, Wv, Wo, bo, W1, b1, W2, b2, g1, be1, g2, be2)
    out = run_bass_kernel_spmd(nc, in_maps, list(range(NCORES)), trace=_trace)
    result = _host_finish(x, bo, b2, xf, out.results)
    if _trace:
        return result, out
    return result
